# revision 3
# baseline (speedup 1.0000x reference)
"""LocalAttention1d Trainium2 kernel.

Layout strategy (B=16 sharded over 8 cores, 2 batches/core):
  - p_t chain in ~fp32 precision: h = tanh(c@W_p.T) via fp16x2 split matmuls
    (c = c1+c2 fp16 pair, W likewise; 3 cross terms give ~1e-7 rel accuracy),
    logit = <tanh(h), V_p> via fused DVE multiply-reduce in fp32.
  - windowed gather: p_int -> int16 row indices -> SWDGE dma_gather straight
    from DRAM q^T (fp16) into (t-partition, 7, 512) tiles.
  - scores: fused DVE multiply-reduce (fp16 2x mode) against u = c@W_a.
  - softmax*gauss -> 7 diagonal fp16 matmuls accumulate the weighted sum in
    PSUM (t-partition layout).
"""

import sys

sys.path.insert(0, "/opt/trn_rl_repo")

import numpy as np

import concourse.bass as bass
import concourse.tile as tile
from concourse import bacc, mybir
from concourse.bass_utils import run_bass_kernel_spmd

B, T, S, QS, CS, PS, D = 16, 1024, 4096, 512, 512, 512, 3
NCORE = 8
BPC = B // NCORE  # batches per core
NJ = 2 * D + 1  # 7 window positions
NT = T // 128  # 8 t-tiles per batch
NIDX = 128 * NJ  # 896 gather indices per t-tile

dt = mybir.dt
AF = mybir.ActivationFunctionType
ALU = mybir.AluOpType

LAST_EXEC_NS = None
LAST_RES = None
_CACHE = {}


def _floor(nc, sp, src, sfx):
    """Exact floor(src) for src >= 0, robust to the cast rounding mode."""
    shp = list(src[:].shape)
    i32 = sp.tile(shp, dt.int32, tag="fli" + sfx)
    nc.vector.tensor_copy(i32[:], src[:])
    cand = sp.tile(shp, dt.float32, tag="flc" + sfx)
    nc.vector.tensor_copy(cand[:], i32[:])
    corr = sp.tile(shp, dt.float32, tag="flx" + sfx)
    nc.vector.scalar_tensor_tensor(
        corr[:], cand[:], 1.0, src[:], ALU.bypass, ALU.is_gt
    )
    res = sp.tile(shp, dt.float32, tag="flr" + sfx)
    nc.vector.tensor_tensor(res[:], cand[:], corr[:], ALU.subtract)
    return res


def _build_nc():
    nc = bacc.Bacc("TRN2", target_bir_lowering=False, debug=False, num_devices=NCORE)

    qT16 = nc.dram_tensor("qT16", [BPC, S, QS], dt.float16, kind="ExternalInput").ap()
    cT1 = nc.dram_tensor("cT1", [BPC, CS, T], dt.float16, kind="ExternalInput").ap()
    cT2 = nc.dram_tensor("cT2", [BPC, CS, T], dt.float16, kind="ExternalInput").ap()
    wp1 = nc.dram_tensor("wp1", [CS, PS], dt.float16, kind="ExternalInput").ap()
    wp2 = nc.dram_tensor("wp2", [CS, PS], dt.float16, kind="ExternalInput").ap()
    wa1 = nc.dram_tensor("wa1", [CS, QS], dt.float16, kind="ExternalInput").ap()
    vpr = nc.dram_tensor("vpr", [128, PS], dt.float32, kind="ExternalInput").ap()
    offs = nc.dram_tensor("offs", [128, NT * NJ], dt.float32, kind="ExternalInput").ap()
    perm8 = nc.dram_tensor("perm8", [128, 8, 128], dt.float32, kind="ExternalInput").ap()
    id128h = nc.dram_tensor("id128h", [128, 128], dt.float16, kind="ExternalInput").ap()
    out = nc.dram_tensor("out", [BPC, T, QS], dt.float32, kind="ExternalOutput").ap()

    with tile.TileContext(nc) as tc:
        import contextlib

        ctx = contextlib.ExitStack()
        with ctx:
            cpool = ctx.enter_context(tc.tile_pool(name="consts", bufs=1))
            ctp = ctx.enter_context(tc.tile_pool(name="ct", bufs=5))
            gp = ctx.enter_context(tc.tile_pool(name="gath", bufs=9))
            up = ctx.enter_context(tc.tile_pool(name="u16", bufs=9))
            sp = ctx.enter_context(tc.tile_pool(name="small", bufs=2))
            jp = ctx.enter_context(tc.tile_pool(name="junk", bufs=2))
            op = ctx.enter_context(tc.tile_pool(name="outp", bufs=2))
            mmp = ctx.enter_context(tc.tile_pool(name="mm", bufs=2, space="PSUM"))
            wsp = ctx.enter_context(tc.tile_pool(name="ws", bufs=2, space="PSUM"))
            tpp = ctx.enter_context(tc.tile_pool(name="tp", bufs=2, space="PSUM"))

            # ---- constants to SBUF (512-row weights folded to (128, 4, N)) ----
            wp1t = cpool.tile([128, 4, PS], dt.float16)
            nc.sync.dma_start(wp1t[:], wp1[:].rearrange("(k p) n -> p k n", p=128))
            wp2t = cpool.tile([128, 4, PS], dt.float16)
            nc.sync.dma_start(wp2t[:], wp2[:].rearrange("(k p) n -> p k n", p=128))
            wa1t = cpool.tile([128, 4, QS], dt.float16)
            nc.sync.dma_start(wa1t[:], wa1[:].rearrange("(k p) n -> p k n", p=128))
            vprt = cpool.tile([128, PS], dt.float32)
            nc.sync.dma_start(vprt[:], vpr[:])
            offst = cpool.tile([128, NT * NJ], dt.float32)
            nc.sync.dma_start(offst[:], offs[:])
            perm8t = cpool.tile([128, 8, 128], dt.float32)
            nc.sync.dma_start(perm8t[:], perm8[:])
            id128ht = cpool.tile([128, 128], dt.float16)
            nc.sync.dma_start(id128ht[:], id128h[:])

            # weight chunk views (k = c-chunk on partitions)
            def chunk(t, k):
                return t[:, k, :]

            for b in range(BPC):
                # ---- load cT halves: 4 chunks of (128, T) each ----
                ct1s, ct2s = [], []
                for k in range(4):
                    c1t = ctp.tile([128, T], dt.float16, tag="ct1")
                    nc.sync.dma_start(c1t[:], cT1[b, k * 128 : (k + 1) * 128, :])
                    ct1s.append(c1t)
                for k in range(4):
                    c2t = ctp.tile([128, T], dt.float16, tag="ct2")
                    nc.sync.dma_start(c2t[:], cT2[b, k * 128 : (k + 1) * 128, :])
                    ct2s.append(c2t)

                logits8 = sp.tile([128, NT], dt.float32, tag="logits8")

                # ---- h (fp16x2: c1W1 + c1W2 + c2W1) + tanh + logit dot ----
                for m in range(NT):
                    hps = mmp.tile([128, PS], dt.float32, tag="hps", space="PSUM")
                    nmm = 0
                    for k in range(4):
                        lhs1 = ct1s[k][:, m * 128 : (m + 1) * 128]
                        lhs2 = ct2s[k][:, m * 128 : (m + 1) * 128]
                        for lhs, rhs in ((lhs1, chunk(wp1t, k)), (lhs1, chunk(wp2t, k)), (lhs2, chunk(wp1t, k))):
                            nc.tensor.matmul(hps[:], lhs, rhs, start=(nmm == 0), stop=(nmm == 11))
                            nmm += 1
                    g = sp.tile([128, PS], dt.float32, tag="g")
                    nc.scalar.activation(g[:], hps[:], AF.Tanh)
                    junkf = jp.tile([128, PS], dt.float32, tag="junkf")
                    nc.vector.scalar_tensor_tensor(
                        junkf[:], g[:], 1.0, vprt[:], ALU.bypass, ALU.mult,
                        accum_out=logits8[:, m : m + 1],
                    )

                # ---- u = c1 @ W_a (fp16) -> u16 ----
                u16s = []
                for m in range(NT):
                    ups = mmp.tile([128, QS], dt.float32, tag="ups", space="PSUM")
                    for k in range(4):
                        nc.tensor.matmul(
                            ups[:], ct1s[k][:, m * 128 : (m + 1) * 128], chunk(wa1t, k),
                            start=(k == 0), stop=(k == 3),
                        )
                    u16 = up.tile([128, QS], dt.float16, tag="u16")
                    nc.scalar.activation(u16[:], ups[:], AF.Copy)
                    u16s.append(u16)

                # ---- index path: fold logits8 into wrapped-16 (128, 64) layout
                # via 8 permutation matmuls: lrep[p, 8m+w] = logits8[w*16+p%16, m]
                lrep = sp.tile([128, 8 * NT], dt.float32, tag="lrep")
                for w in range(8):
                    pps = tpp.tile([128, NT], dt.float32, tag="pps", space="PSUM")
                    nc.tensor.matmul(
                        pps[:], perm8t[:, w, :], logits8[:], start=True, stop=True
                    )
                    nc.vector.tensor_copy(
                        lrep[:].rearrange("p (m w) -> p w m", w=8)[:, w, :], pps[:]
                    )
                s2 = sp.tile([128, 8 * NT], dt.float32, tag="s2")
                nc.scalar.activation(s2[:], lrep[:], AF.Sigmoid)
                ps2 = sp.tile([128, 8 * NT], dt.float32, tag="ps2")
                nc.vector.tensor_scalar_mul(ps2[:], s2[:], 4096.0)
                pi2 = _floor(nc, sp, ps2, "2")
                idxs = sp.tile([128, NT * 56], dt.int16, tag="idxs")
                idx3 = idxs[:].rearrange("p (m j w) -> p m j w", m=NT, j=NJ, w=8)
                tmpp = sp.tile([128, 8 * NT], dt.float32, tag="tmpp")
                pi2v = pi2[:].rearrange("p (m w) -> p m w", m=NT, w=8)
                for j in range(NJ):
                    nc.vector.tensor_scalar(
                        tmpp[:], pi2[:], float(j - 3), 0.0, ALU.add, ALU.max
                    )
                    nc.vector.tensor_scalar(
                        idx3[:, :, j, :], tmpp[:].rearrange("p (m w) -> p m w", m=NT, w=8),
                        4095.0, None, ALU.min,
                    )

                # ---- t-partition p values: sigma, p_t, p_int, gauss, mask ----
                sig8 = sp.tile([128, NT], dt.float32, tag="sig8")
                nc.scalar.activation(sig8[:], logits8[:], AF.Sigmoid)
                pt8 = sp.tile([128, NT], dt.float32, tag="pt8")
                nc.vector.tensor_scalar_mul(pt8[:], sig8[:], 4096.0)
                pi8 = _floor(nc, sp, pt8, "8")

                NW = NT * NJ
                pos_all = sp.tile([128, NW], dt.float32, tag="pos_all")
                pos3 = pos_all[:].rearrange("p (m j) -> p m j", j=NJ)
                nc.vector.scalar_tensor_tensor(
                    pos3, pi8[:, :, None].broadcast_to([128, NT, NJ]), 1.0,
                    offst[:].rearrange("p (m j) -> p m j", j=NJ),
                    ALU.bypass, ALU.add,
                )
                dtile = sp.tile([128, NW], dt.float32, tag="dtile")
                nc.vector.scalar_tensor_tensor(
                    dtile[:].rearrange("p (m j) -> p m j", j=NJ),
                    pt8[:, :, None].broadcast_to([128, NT, NJ]), 1.0,
                    pos3, ALU.bypass, ALU.subtract,
                )
                g1 = sp.tile([128, NW], dt.float32, tag="g1")
                nc.scalar.activation(g1[:], dtile[:], AF.Square, scale=float(np.sqrt(2.0) / 3.0))
                gauss = sp.tile([128, NW], dt.float32, tag="gauss")
                nc.scalar.activation(gauss[:], g1[:], AF.Exp, scale=-1.0)
                m1 = sp.tile([128, NW], dt.float32, tag="m1")
                nc.vector.tensor_scalar(m1[:], pos_all[:], 0.0, -1e30, ALU.is_lt, ALU.mult)
                maskb = sp.tile([128, NW], dt.float32, tag="maskb")
                nc.vector.tensor_scalar(maskb[:], pos_all[:], 4095.0, -1e30, ALU.is_gt, ALU.mult)
                nc.vector.tensor_add(maskb[:], maskb[:], m1[:])

                # ---- gathers + scores ----
                a_all = sp.tile([128, NW], dt.float32, tag="a_all")
                gts = []
                for m in range(NT):
                    gt = gp.tile([128, NJ, QS], dt.float16, tag="gt")
                    nc.gpsimd.dma_gather(
                        gt[:], qT16[b], idxs[:, m * 56 : (m + 1) * 56], NIDX, NIDX, QS,
                        single_packet=False,
                    )
                    gts.append(gt)
                    for j in range(NJ):
                        junk16 = jp.tile([128, QS], dt.float16, tag="junk16")
                        nc.vector.scalar_tensor_tensor(
                            junk16[:], gt[:, j, :], 1.0, u16s[m][:],
                            ALU.bypass, ALU.mult,
                            accum_out=a_all[:, m * NJ + j : m * NJ + j + 1],
                        )

                # ---- batched masked softmax * gauss ----
                nc.vector.tensor_add(a_all[:], a_all[:], maskb[:])
                a3 = a_all[:].rearrange("p (m j) -> p m j", j=NJ)
                rmax = sp.tile([128, NT], dt.float32, tag="rmax")
                nc.vector.tensor_reduce(rmax[:, :, None], a3, mybir.AxisListType.X, ALU.max)
                asub = sp.tile([128, NW], dt.float32, tag="asub")
                nc.vector.scalar_tensor_tensor(
                    asub[:].rearrange("p (m j) -> p m j", j=NJ),
                    rmax[:, :, None].broadcast_to([128, NT, NJ]), 1.0,
                    a3, ALU.bypass, ALU.subtract,
                )
                e_all = sp.tile([128, NW], dt.float32, tag="e_all")
                nc.scalar.activation(e_all[:], asub[:], AF.Exp, scale=-1.0)
                rsum = sp.tile([128, NT], dt.float32, tag="rsum")
                nc.vector.tensor_reduce(
                    rsum[:, :, None], e_all[:].rearrange("p (m j) -> p m j", j=NJ),
                    mybir.AxisListType.X, ALU.add,
                )
                rinv = sp.tile([128, NT], dt.float32, tag="rinv")
                nc.vector.reciprocal(rinv[:], rsum[:])
                wt = sp.tile([128, NW], dt.float32, tag="wt")
                nc.vector.scalar_tensor_tensor(
                    wt[:].rearrange("p (m j) -> p m j", j=NJ),
                    rinv[:, :, None].broadcast_to([128, NT, NJ]), 1.0,
                    e_all[:].rearrange("p (m j) -> p m j", j=NJ),
                    ALU.bypass, ALU.mult,
                )
                nc.vector.tensor_mul(wt[:], wt[:], gauss[:])
                wt16 = sp.tile([128, NW], dt.float16, tag="wt16")
                nc.vector.tensor_copy(wt16[:], wt[:])

                # ---- weighted sum via diagonal fp16 matmuls ----
                for m in range(NT):
                    dall = sp.tile([128, NJ * 128], dt.float16, tag="dall")
                    nc.vector.tensor_tensor(
                        dall[:].rearrange("p (j q) -> p j q", j=NJ),
                        id128ht[:, None, :].broadcast_to([128, NJ, 128]),
                        wt16[:, m * NJ : (m + 1) * NJ][:, :, None].broadcast_to([128, NJ, 128]),
                        ALU.mult,
                    )
                    wps = wsp.tile([128, QS], dt.float32, tag="wps", space="PSUM")
                    for j in range(NJ):
                        nc.tensor.matmul(
                            wps[:], dall[:, j * 128 : (j + 1) * 128], gts[m][:, j, :],
                            start=(j == 0), stop=(j == NJ - 1),
                        )
                    outt = op.tile([128, QS], dt.float32, tag="outt")
                    nc.scalar.activation(outt[:], wps[:], AF.Copy)
                    nc.sync.dma_start(out[b, m * 128 : (m + 1) * 128, :], outt[:])

    nc.compile()
    return nc


def _host_prep(q, c_t, W_a, W_p, V_p):
    q = np.asarray(q, dtype=np.float32)
    c_t = np.asarray(c_t, dtype=np.float32)
    W_a = np.asarray(W_a, dtype=np.float32)
    W_p = np.asarray(W_p, dtype=np.float32)
    V_p = np.asarray(V_p, dtype=np.float32)

    qT16 = np.ascontiguousarray(q.transpose(0, 2, 1)).astype(np.float16)
    cT = np.ascontiguousarray(c_t.transpose(0, 2, 1))
    cT1 = cT.astype(np.float16)
    cT2 = (cT - cT1.astype(np.float32)).astype(np.float16)
    wpT = np.ascontiguousarray(W_p.T)
    wp1 = wpT.astype(np.float16)
    wp2 = (wpT - wp1.astype(np.float32)).astype(np.float16)
    wa1 = W_a.astype(np.float16)
    vpr = np.ascontiguousarray(np.tile(V_p.reshape(1, PS), (128, 1)), dtype=np.float32)
    offs = np.tile(np.arange(-3, 4, dtype=np.float32).reshape(1, 1, NJ), (128, NT, 1))
    offs = np.ascontiguousarray(offs.reshape(128, NT * NJ))
    perm8 = np.zeros((128, 8, 128), dtype=np.float32)
    for w in range(8):
        for p in range(128):
            perm8[w * 16 + p % 16, w, p] = 1.0
    id128h = np.eye(128).astype(np.float16)

    consts = dict(wp1=wp1, wp2=wp2, wa1=wa1, vpr=vpr, offs=offs, perm8=perm8,
                  id128h=id128h)
    in_maps = []
    for k in range(NCORE):
        sl = slice(k * BPC, (k + 1) * BPC)
        m = dict(consts)
        m["qT16"] = np.ascontiguousarray(qT16[sl])
        m["cT1"] = np.ascontiguousarray(cT1[sl])
        m["cT2"] = np.ascontiguousarray(cT2[sl])
        in_maps.append(m)
    return in_maps


def kernel(q, c_t, W_a, W_p, V_p):
    global LAST_EXEC_NS, LAST_RES
    if "nc" not in _CACHE:
        _CACHE["nc"] = _build_nc()
    nc = _CACHE["nc"]
    in_maps = _host_prep(q, c_t, W_a, W_p, V_p)
    res = run_bass_kernel_spmd(nc, in_maps, core_ids=list(range(NCORE)))
    LAST_RES = res
    LAST_EXEC_NS = res.exec_time_ns
    outs = [res.results[k]["out"] for k in range(NCORE)]
    return np.concatenate(outs, axis=0).astype(np.float32)



# revision 6
# speedup vs baseline: 1.5915x; 1.5915x over previous
"""LocalAttention1d Trainium2 kernel.

Layout strategy (B=16 sharded over 8 cores, 2 batches/core):
  - p_t chain in ~fp32 precision: h = tanh(c@W_p.T) via fp16x2 split matmuls
    (c = c1+c2 fp16 pair, W likewise; 3 cross terms give ~1e-7 rel accuracy),
    logit = <tanh(h), V_p> via fused DVE multiply-reduce in fp32.
  - windowed gather: p_int -> int16 row indices -> SWDGE dma_gather straight
    from DRAM q^T (fp16) into (t-partition, 7, 512) tiles.
  - scores: fused DVE multiply-reduce (fp16 2x mode) against u = c@W_a.
  - softmax*gauss -> 7 diagonal fp16 matmuls accumulate the weighted sum in
    PSUM (t-partition layout).
"""

import sys

sys.path.insert(0, "/opt/trn_rl_repo")

import numpy as np

import bass_rust
import concourse.bass as bass
import concourse.tile as tile
from concourse import bacc, mybir
from concourse.bass_utils import run_bass_kernel_spmd

B, T, S, QS, CS, PS, D = 16, 1024, 4096, 512, 512, 512, 3
NCORE = 8
BPC = B // NCORE  # batches per core
NJ = 2 * D + 1  # 7 window positions
NT = T // 128  # 8 t-tiles per batch
NIDX = 128 * NJ  # 896 gather indices per t-tile

dt = mybir.dt
AF = mybir.ActivationFunctionType
ALU = mybir.AluOpType

LAST_EXEC_NS = None
LAST_RES = None
_CACHE = {}


def _floor(nc, sp, src, sfx):
    """Exact floor(src) for src >= 0, robust to the cast rounding mode."""
    shp = list(src[:].shape)
    i32 = sp.tile(shp, dt.int32, tag="fli" + sfx)
    nc.vector.tensor_copy(i32[:], src[:])
    cand = sp.tile(shp, dt.float32, tag="flc" + sfx)
    nc.vector.tensor_copy(cand[:], i32[:])
    corr = sp.tile(shp, dt.float32, tag="flx" + sfx)
    nc.vector.scalar_tensor_tensor(
        corr[:], cand[:], 1.0, src[:], ALU.bypass, ALU.is_gt
    )
    res = sp.tile(shp, dt.float32, tag="flr" + sfx)
    nc.vector.tensor_tensor(res[:], cand[:], corr[:], ALU.subtract)
    return res


def _build_nc():
    nc = bacc.Bacc("TRN2", target_bir_lowering=False, debug=False, num_devices=NCORE)

    qT16 = nc.dram_tensor("qT16", [BPC, S, QS], dt.float16, kind="ExternalInput").ap()
    cT1 = nc.dram_tensor("cT1", [BPC, CS, T], dt.float16, kind="ExternalInput").ap()
    cT2 = nc.dram_tensor("cT2", [BPC, CS, T], dt.float16, kind="ExternalInput").ap()
    wp1 = nc.dram_tensor("wp1", [CS, PS], dt.float16, kind="ExternalInput").ap()
    wp2 = nc.dram_tensor("wp2", [CS, PS], dt.float16, kind="ExternalInput").ap()
    wa1 = nc.dram_tensor("wa1", [CS, QS], dt.float16, kind="ExternalInput").ap()
    vpr = nc.dram_tensor("vpr", [128, PS], dt.float32, kind="ExternalInput").ap()
    offs = nc.dram_tensor("offs", [128, NT * NJ], dt.float32, kind="ExternalInput").ap()
    perm8 = nc.dram_tensor("perm8", [128, 8, 128], dt.float32, kind="ExternalInput").ap()
    id128h = nc.dram_tensor("id128h", [128, 128], dt.float16, kind="ExternalInput").ap()
    out = nc.dram_tensor("out", [BPC, T, QS], dt.float32, kind="ExternalOutput").ap()

    with tile.TileContext(nc) as tc:
        import contextlib

        ctx = contextlib.ExitStack()
        with ctx:
            cpool = ctx.enter_context(tc.tile_pool(name="consts", bufs=1))
            ctp = ctx.enter_context(tc.tile_pool(name="ct", bufs=5))
            gp = ctx.enter_context(tc.tile_pool(name="gath", bufs=9))
            up = ctx.enter_context(tc.tile_pool(name="u16", bufs=9))
            sp = ctx.enter_context(tc.tile_pool(name="small", bufs=2))
            jp = ctx.enter_context(tc.tile_pool(name="junk", bufs=2))
            op = ctx.enter_context(tc.tile_pool(name="outp", bufs=2))
            mmp = ctx.enter_context(tc.tile_pool(name="mm", bufs=2, space="PSUM"))
            wsp = ctx.enter_context(tc.tile_pool(name="ws", bufs=2, space="PSUM"))
            tpp = ctx.enter_context(tc.tile_pool(name="tp", bufs=2, space="PSUM"))

            # ---- constants to SBUF (512-row weights folded to (128, 4, N)) ----
            wp1t = cpool.tile([128, 4, PS], dt.float16)
            nc.sync.dma_start(wp1t[:], wp1[:].rearrange("(k p) n -> p k n", p=128))
            wp2t = cpool.tile([128, 4, PS], dt.float16)
            nc.sync.dma_start(wp2t[:], wp2[:].rearrange("(k p) n -> p k n", p=128))
            wa1t = cpool.tile([128, 4, QS], dt.float16)
            nc.sync.dma_start(wa1t[:], wa1[:].rearrange("(k p) n -> p k n", p=128))
            vprt = cpool.tile([128, PS], dt.float32)
            nc.sync.dma_start(vprt[:], vpr[:])
            offst = cpool.tile([128, NT * NJ], dt.float32)
            nc.sync.dma_start(offst[:], offs[:])
            perm8t = cpool.tile([128, 8, 128], dt.float32)
            nc.sync.dma_start(perm8t[:], perm8[:])
            id128ht = cpool.tile([128, 128], dt.float16)
            nc.sync.dma_start(id128ht[:], id128h[:])

            # weight chunk views (k = c-chunk on partitions)
            def chunk(t, k):
                return t[:, k, :]

            for b in range(BPC):
                # ---- load cT halves: 4 chunks of (128, T) each ----
                ct1s, ct2s = [], []
                for k in range(4):
                    c1t = ctp.tile([128, T], dt.float16, tag="ct1")
                    nc.sync.dma_start(c1t[:], cT1[b, k * 128 : (k + 1) * 128, :])
                    ct1s.append(c1t)
                for k in range(4):
                    c2t = ctp.tile([128, T], dt.float16, tag="ct2")
                    nc.sync.dma_start(c2t[:], cT2[b, k * 128 : (k + 1) * 128, :])
                    ct2s.append(c2t)

                logits8 = sp.tile([128, NT], dt.float32, tag="logits8")

                # ---- h (fp16x2: c1W1 + c1W2 + c2W1) + tanh + logit dot ----
                for m in range(NT):
                    hps = mmp.tile([128, PS], dt.float32, tag="hps", space="PSUM")
                    nmm = 0
                    for k in range(4):
                        lhs1 = ct1s[k][:, m * 128 : (m + 1) * 128]
                        lhs2 = ct2s[k][:, m * 128 : (m + 1) * 128]
                        for lhs, rhs in ((lhs1, chunk(wp1t, k)), (lhs1, chunk(wp2t, k)), (lhs2, chunk(wp1t, k))):
                            nc.tensor.matmul(hps[:], lhs, rhs, start=(nmm == 0), stop=(nmm == 11))
                            nmm += 1
                    g = sp.tile([128, PS], dt.float32, tag="g")
                    nc.scalar.activation(g[:], hps[:], AF.Tanh)
                    junkf = jp.tile([128, PS], dt.float32, tag="junkf")
                    nc.vector.scalar_tensor_tensor(
                        junkf[:], g[:], 1.0, vprt[:], ALU.bypass, ALU.mult,
                        accum_out=logits8[:, m : m + 1],
                    )

                # ---- u = c1 @ W_a (fp16) -> u16 ----
                u16s = []
                for m in range(NT):
                    ups = mmp.tile([128, QS], dt.float32, tag="ups", space="PSUM")
                    for k in range(4):
                        nc.tensor.matmul(
                            ups[:], ct1s[k][:, m * 128 : (m + 1) * 128], chunk(wa1t, k),
                            start=(k == 0), stop=(k == 3),
                        )
                    u16 = up.tile([128, QS], dt.float16, tag="u16")
                    nc.scalar.activation(u16[:], ups[:], AF.Copy)
                    u16s.append(u16)

                # ---- index path: fold logits8 into wrapped-16 (128, 64) layout
                # via 8 permutation matmuls: lrep[p, 8m+w] = logits8[w*16+p%16, m]
                lrep = sp.tile([128, 8 * NT], dt.float32, tag="lrep")
                for w in range(8):
                    pps = tpp.tile([128, NT], dt.float32, tag="pps", space="PSUM")
                    nc.tensor.matmul(
                        pps[:], perm8t[:, w, :], logits8[:], start=True, stop=True
                    )
                    nc.vector.tensor_copy(
                        lrep[:].rearrange("p (m w) -> p w m", w=8)[:, w, :], pps[:]
                    )
                s2 = sp.tile([128, 8 * NT], dt.float32, tag="s2")
                nc.scalar.activation(s2[:], lrep[:], AF.Sigmoid)
                ps2 = sp.tile([128, 8 * NT], dt.float32, tag="ps2")
                nc.vector.tensor_scalar_mul(ps2[:], s2[:], 4096.0)
                pi2 = _floor(nc, sp, ps2, "2")
                # single block-start index per t: clamp(p_int, 3, 4092) - 3
                idxs = sp.tile([128, NT, 8], dt.int16, tag="idxs")
                tmpp = sp.tile([128, 8 * NT], dt.float32, tag="tmpp")
                nc.vector.tensor_scalar(
                    tmpp[:], pi2[:], 3.0, 4092.0, ALU.max, ALU.min
                )
                nc.vector.tensor_scalar(
                    idxs[:].rearrange("p m w -> p (m w)"), tmpp[:], -3.0, None, ALU.add
                )

                # ---- t-partition p values: sigma, p_t, p_int, gauss, mask ----
                sig8 = sp.tile([128, NT], dt.float32, tag="sig8")
                nc.scalar.activation(sig8[:], logits8[:], AF.Sigmoid)
                pt8 = sp.tile([128, NT], dt.float32, tag="pt8")
                nc.vector.tensor_scalar_mul(pt8[:], sig8[:], 4096.0)
                pi8 = _floor(nc, sp, pt8, "8")

                NW = NT * NJ
                pos_all = sp.tile([128, NW], dt.float32, tag="pos_all")
                pos3 = pos_all[:].rearrange("p (m j) -> p m j", j=NJ)
                nc.vector.scalar_tensor_tensor(
                    pos3, pi8[:, :, None].broadcast_to([128, NT, NJ]), 1.0,
                    offst[:].rearrange("p (m j) -> p m j", j=NJ),
                    ALU.bypass, ALU.add,
                )
                dtile = sp.tile([128, NW], dt.float32, tag="dtile")
                nc.vector.scalar_tensor_tensor(
                    dtile[:].rearrange("p (m j) -> p m j", j=NJ),
                    pt8[:, :, None].broadcast_to([128, NT, NJ]), 1.0,
                    pos3, ALU.bypass, ALU.subtract,
                )
                g1 = sp.tile([128, NW], dt.float32, tag="g1")
                nc.scalar.activation(g1[:], dtile[:], AF.Square, scale=float(np.sqrt(2.0) / 3.0))
                gauss = sp.tile([128, NW], dt.float32, tag="gauss")
                nc.scalar.activation(gauss[:], g1[:], AF.Exp, scale=-1.0)
                m1 = sp.tile([128, NW], dt.float32, tag="m1")
                nc.vector.tensor_scalar(m1[:], pos_all[:], 0.0, -1e30, ALU.is_lt, ALU.mult)
                maskb = sp.tile([128, NW], dt.float32, tag="maskb")
                nc.vector.tensor_scalar(maskb[:], pos_all[:], 4095.0, -1e30, ALU.is_gt, ALU.mult)
                nc.vector.tensor_add(maskb[:], maskb[:], m1[:])

                # ---- gathers (one 7-row block descriptor per t) + scores ----
                qwin = qT16[b].copy()
                qwin.ap = bass_rust.VecI64Pair([[QS, S - NJ + 1], [1, NJ * QS]])
                a_all = sp.tile([128, NW], dt.float32, tag="a_all")
                gts = []
                for m in range(NT):
                    gt = gp.tile([128, 1, NJ * QS], dt.float16, tag="gt")
                    nc.gpsimd.dma_gather(
                        gt[:], qwin, idxs[:, m, :], 128, 128, NJ * QS,
                        elem_step=QS, single_packet=False,
                    )
                    gtv = gt[:, 0, :].rearrange("p (j q) -> p j q", j=NJ)
                    gts.append(gtv)
                    for j in range(NJ):
                        junk16 = jp.tile([128, QS], dt.float16, tag="junk16")
                        nc.vector.scalar_tensor_tensor(
                            junk16[:], gtv[:, j, :], 1.0, u16s[m][:],
                            ALU.bypass, ALU.mult,
                            accum_out=a_all[:, m * NJ + j : m * NJ + j + 1],
                        )

                # ---- batched masked softmax * gauss ----
                nc.vector.tensor_add(a_all[:], a_all[:], maskb[:])
                a3 = a_all[:].rearrange("p (m j) -> p m j", j=NJ)
                rmax = sp.tile([128, NT], dt.float32, tag="rmax")
                nc.vector.tensor_reduce(rmax[:, :, None], a3, mybir.AxisListType.X, ALU.max)
                asub = sp.tile([128, NW], dt.float32, tag="asub")
                nc.vector.scalar_tensor_tensor(
                    asub[:].rearrange("p (m j) -> p m j", j=NJ),
                    rmax[:, :, None].broadcast_to([128, NT, NJ]), 1.0,
                    a3, ALU.bypass, ALU.subtract,
                )
                e_all = sp.tile([128, NW], dt.float32, tag="e_all")
                nc.scalar.activation(e_all[:], asub[:], AF.Exp, scale=-1.0)
                rsum = sp.tile([128, NT], dt.float32, tag="rsum")
                nc.vector.tensor_reduce(
                    rsum[:, :, None], e_all[:].rearrange("p (m j) -> p m j", j=NJ),
                    mybir.AxisListType.X, ALU.add,
                )
                rinv = sp.tile([128, NT], dt.float32, tag="rinv")
                nc.vector.reciprocal(rinv[:], rsum[:])
                wt = sp.tile([128, NW], dt.float32, tag="wt")
                nc.vector.scalar_tensor_tensor(
                    wt[:].rearrange("p (m j) -> p m j", j=NJ),
                    rinv[:, :, None].broadcast_to([128, NT, NJ]), 1.0,
                    e_all[:].rearrange("p (m j) -> p m j", j=NJ),
                    ALU.bypass, ALU.mult,
                )
                nc.vector.tensor_mul(wt[:], wt[:], gauss[:])
                wt16 = sp.tile([128, NW], dt.float16, tag="wt16")
                nc.vector.tensor_copy(wt16[:], wt[:])

                # ---- weighted sum via diagonal fp16 matmuls ----
                for m in range(NT):
                    dall = sp.tile([128, NJ * 128], dt.float16, tag="dall")
                    nc.vector.tensor_tensor(
                        dall[:].rearrange("p (j q) -> p j q", j=NJ),
                        id128ht[:, None, :].broadcast_to([128, NJ, 128]),
                        wt16[:, m * NJ : (m + 1) * NJ][:, :, None].broadcast_to([128, NJ, 128]),
                        ALU.mult,
                    )
                    wps = wsp.tile([128, QS], dt.float32, tag="wps", space="PSUM")
                    for j in range(NJ):
                        nc.tensor.matmul(
                            wps[:], dall[:, j * 128 : (j + 1) * 128], gts[m][:, j, :],
                            start=(j == 0), stop=(j == NJ - 1),
                        )
                    outt = op.tile([128, QS], dt.float32, tag="outt")
                    nc.scalar.activation(outt[:], wps[:], AF.Copy)
                    nc.sync.dma_start(out[b, m * 128 : (m + 1) * 128, :], outt[:])

    nc.compile()
    return nc


def _host_prep(q, c_t, W_a, W_p, V_p):
    q = np.asarray(q, dtype=np.float32)
    c_t = np.asarray(c_t, dtype=np.float32)
    W_a = np.asarray(W_a, dtype=np.float32)
    W_p = np.asarray(W_p, dtype=np.float32)
    V_p = np.asarray(V_p, dtype=np.float32)

    qT16 = np.ascontiguousarray(q.transpose(0, 2, 1)).astype(np.float16)
    cT = np.ascontiguousarray(c_t.transpose(0, 2, 1))
    cT1 = cT.astype(np.float16)
    cT2 = (cT - cT1.astype(np.float32)).astype(np.float16)
    wpT = np.ascontiguousarray(W_p.T)
    wp1 = wpT.astype(np.float16)
    wp2 = (wpT - wp1.astype(np.float32)).astype(np.float16)
    wa1 = W_a.astype(np.float16)
    vpr = np.ascontiguousarray(np.tile(V_p.reshape(1, PS), (128, 1)), dtype=np.float32)
    offs = np.tile(np.arange(-3, 4, dtype=np.float32).reshape(1, 1, NJ), (128, NT, 1))
    offs = np.ascontiguousarray(offs.reshape(128, NT * NJ))
    perm8 = np.zeros((128, 8, 128), dtype=np.float32)
    for w in range(8):
        for p in range(128):
            perm8[w * 16 + p % 16, w, p] = 1.0
    id128h = np.eye(128).astype(np.float16)

    consts = dict(wp1=wp1, wp2=wp2, wa1=wa1, vpr=vpr, offs=offs, perm8=perm8,
                  id128h=id128h)
    in_maps = []
    for k in range(NCORE):
        sl = slice(k * BPC, (k + 1) * BPC)
        m = dict(consts)
        m["qT16"] = np.ascontiguousarray(qT16[sl])
        m["cT1"] = np.ascontiguousarray(cT1[sl])
        m["cT2"] = np.ascontiguousarray(cT2[sl])
        in_maps.append(m)
    return in_maps


def kernel(q, c_t, W_a, W_p, V_p):
    global LAST_EXEC_NS, LAST_RES
    if "nc" not in _CACHE:
        _CACHE["nc"] = _build_nc()
    nc = _CACHE["nc"]
    in_maps = _host_prep(q, c_t, W_a, W_p, V_p)
    res = run_bass_kernel_spmd(nc, in_maps, core_ids=list(range(NCORE)))
    LAST_RES = res
    LAST_EXEC_NS = res.exec_time_ns
    outs = [res.results[k]["out"] for k in range(NCORE)]
    return np.concatenate(outs, axis=0).astype(np.float32)



# revision 10
# speedup vs baseline: 1.6000x; 1.0054x over previous
"""LocalAttention1d Trainium2 kernel.

Layout strategy (B=16 sharded over 8 cores, 2 batches/core):
  - p_t chain in ~fp32 precision: h = tanh(c@W_p.T) via fp16x2 split matmuls
    (c = c1+c2 fp16 pair, W likewise; 3 cross terms give ~1e-7 rel accuracy),
    logit = <tanh(h), V_p> via fused DVE multiply-reduce in fp32.
  - windowed gather: p_int -> int16 row indices -> SWDGE dma_gather straight
    from DRAM q^T (fp16) into (t-partition, 7, 512) tiles.
  - scores: fused DVE multiply-reduce (fp16 2x mode) against u = c@W_a.
  - softmax*gauss -> 7 diagonal fp16 matmuls accumulate the weighted sum in
    PSUM (t-partition layout).
"""

import sys

sys.path.insert(0, "/opt/trn_rl_repo")

import numpy as np

import bass_rust
import concourse.bass as bass
import concourse.tile as tile
from concourse import bacc, mybir
from concourse.bass_utils import run_bass_kernel_spmd

B, T, S, QS, CS, PS, D = 16, 1024, 4096, 512, 512, 512, 3
NCORE = 8
BPC = B // NCORE  # batches per core
NJ = 2 * D + 1  # 7 window positions
NT = T // 128  # 8 t-tiles per batch
NIDX = 128 * NJ  # 896 gather indices per t-tile

dt = mybir.dt
AF = mybir.ActivationFunctionType
ALU = mybir.AluOpType

LAST_EXEC_NS = None
LAST_RES = None
_CACHE = {}


def _floor(nc, sp, src, sfx):
    """Exact floor(src) for src >= 0, robust to the cast rounding mode."""
    shp = list(src[:].shape)
    i32 = sp.tile(shp, dt.int32, tag="fli" + sfx)
    nc.vector.tensor_copy(i32[:], src[:])
    cand = sp.tile(shp, dt.float32, tag="flc" + sfx)
    nc.vector.tensor_copy(cand[:], i32[:])
    corr = sp.tile(shp, dt.float32, tag="flx" + sfx)
    nc.vector.scalar_tensor_tensor(
        corr[:], cand[:], 1.0, src[:], ALU.bypass, ALU.is_gt
    )
    res = sp.tile(shp, dt.float32, tag="flr" + sfx)
    nc.vector.tensor_tensor(res[:], cand[:], corr[:], ALU.subtract)
    return res


def _build_nc():
    nc = bacc.Bacc("TRN2", target_bir_lowering=False, debug=False, num_devices=NCORE)

    qT16 = nc.dram_tensor("qT16", [BPC, S, QS], dt.float16, kind="ExternalInput").ap()
    cT1 = nc.dram_tensor("cT1", [BPC, CS, T], dt.float16, kind="ExternalInput").ap()
    cT2 = nc.dram_tensor("cT2", [BPC, CS, T], dt.float16, kind="ExternalInput").ap()
    wp1 = nc.dram_tensor("wp1", [CS, PS], dt.float16, kind="ExternalInput").ap()
    wp2 = nc.dram_tensor("wp2", [CS, PS], dt.float16, kind="ExternalInput").ap()
    wa1 = nc.dram_tensor("wa1", [CS, QS], dt.float16, kind="ExternalInput").ap()
    vpr = nc.dram_tensor("vpr", [128, PS], dt.float32, kind="ExternalInput").ap()
    offs = nc.dram_tensor("offs", [128, NT * NJ], dt.float32, kind="ExternalInput").ap()
    perm8 = nc.dram_tensor("perm8", [128, 8, 128], dt.float32, kind="ExternalInput").ap()
    id128h = nc.dram_tensor("id128h", [128, 128], dt.float16, kind="ExternalInput").ap()
    out = nc.dram_tensor("out", [BPC, T, QS], dt.float16, kind="ExternalOutput").ap()

    with tile.TileContext(nc) as tc:
        import contextlib

        ctx = contextlib.ExitStack()
        with ctx:
            cpool = ctx.enter_context(tc.tile_pool(name="consts", bufs=1))
            ctp = ctx.enter_context(tc.tile_pool(name="ct", bufs=16))
            gp = ctx.enter_context(tc.tile_pool(name="gath", bufs=9))
            up = ctx.enter_context(tc.tile_pool(name="u16", bufs=9))
            sp = ctx.enter_context(tc.tile_pool(name="small", bufs=2))
            jp = ctx.enter_context(tc.tile_pool(name="junk", bufs=2))
            op = ctx.enter_context(tc.tile_pool(name="outp", bufs=2))
            mmp = ctx.enter_context(tc.tile_pool(name="mm", bufs=2, space="PSUM"))
            wsp = ctx.enter_context(tc.tile_pool(name="ws", bufs=2, space="PSUM"))
            tpp = ctx.enter_context(tc.tile_pool(name="tp", bufs=2, space="PSUM"))

            # ---- constants to SBUF (512-row weights folded to (128, 4, N)) ----
            # order matters: first h matmul needs only wp1t/wp2t + c1 chunk 0
            wp1t = cpool.tile([128, 4, PS], dt.float16)
            nc.sync.dma_start(wp1t[:], wp1[:].rearrange("(k p) n -> p k n", p=128))
            wp2t = cpool.tile([128, 4, PS], dt.float16)
            nc.sync.dma_start(wp2t[:], wp2[:].rearrange("(k p) n -> p k n", p=128))

            # c loads on the scalar queue so they don't serialize behind consts
            all_ct1s, all_ct2s = [], []
            for b in range(BPC):
                ct1s, ct2s = [], []
                for k in range(4):
                    c1t = ctp.tile([128, T], dt.float16, tag="ct1")
                    nc.scalar.dma_start(c1t[:], cT1[b, k * 128 : (k + 1) * 128, :])
                    ct1s.append(c1t)
                for k in range(4):
                    c2t = ctp.tile([128, T], dt.float16, tag="ct2")
                    nc.scalar.dma_start(c2t[:], cT2[b, k * 128 : (k + 1) * 128, :])
                    ct2s.append(c2t)
                all_ct1s.append(ct1s)
                all_ct2s.append(ct2s)

            wa1t = cpool.tile([128, 4, QS], dt.float16)
            nc.sync.dma_start(wa1t[:], wa1[:].rearrange("(k p) n -> p k n", p=128))
            vprt = cpool.tile([128, PS], dt.float32)
            nc.sync.dma_start(vprt[:], vpr[:])
            offst = cpool.tile([128, NT * NJ], dt.float32)
            nc.sync.dma_start(offst[:], offs[:])
            perm8t = cpool.tile([128, 8, 128], dt.float32)
            nc.sync.dma_start(perm8t[:], perm8[:])
            id128ht = cpool.tile([128, 128], dt.float16)
            nc.sync.dma_start(id128ht[:], id128h[:])

            # weight chunk views (k = c-chunk on partitions)
            def chunk(t, k):
                return t[:, k, :]

            for b in range(BPC):
                ct1s, ct2s = all_ct1s[b], all_ct2s[b]

                logits8 = sp.tile([128, NT], dt.float32, tag="logits8")

                # ---- h (fp16x2: c1W1 + c1W2 + c2W1) + tanh + logit dot ----
                # c1 terms first so compute starts before c2 chunks land
                for m in range(NT):
                    hps = mmp.tile([128, PS], dt.float32, tag="hps", space="PSUM")
                    nmm = 0
                    terms = [
                        (ct1s[k][:, m * 128 : (m + 1) * 128], chunk(w, k))
                        for k in range(4)
                        for w in (wp1t, wp2t)
                    ] + [
                        (ct2s[k][:, m * 128 : (m + 1) * 128], chunk(wp1t, k))
                        for k in range(4)
                    ]
                    for lhs, rhs in terms:
                        nc.tensor.matmul(hps[:], lhs, rhs, start=(nmm == 0), stop=(nmm == 11))
                        nmm += 1
                    g = sp.tile([128, PS], dt.float32, tag="g")
                    nc.scalar.activation(g[:], hps[:], AF.Tanh)
                    junkf = jp.tile([128, PS], dt.float32, tag="junkf")
                    nc.vector.scalar_tensor_tensor(
                        junkf[:], g[:], 1.0, vprt[:], ALU.bypass, ALU.mult,
                        accum_out=logits8[:, m : m + 1],
                    )

                # ---- u = c1 @ W_a (fp16) -> u16 ----
                u16s = []
                for m in range(NT):
                    ups = mmp.tile([128, QS], dt.float32, tag="ups", space="PSUM")
                    for k in range(4):
                        nc.tensor.matmul(
                            ups[:], ct1s[k][:, m * 128 : (m + 1) * 128], chunk(wa1t, k),
                            start=(k == 0), stop=(k == 3),
                        )
                    u16 = up.tile([128, QS], dt.float16, tag="u16")
                    nc.scalar.activation(u16[:], ups[:], AF.Copy)
                    u16s.append(u16)

                # ---- index path: fold logits8 into wrapped-16 (128, 64) layout
                # via 8 permutation matmuls: lrep[p, 8m+w] = logits8[w*16+p%16, m]
                lrep = sp.tile([128, 8 * NT], dt.float32, tag="lrep")
                for w in range(8):
                    pps = tpp.tile([128, NT], dt.float32, tag="pps", space="PSUM")
                    nc.tensor.matmul(
                        pps[:], perm8t[:, w, :], logits8[:], start=True, stop=True
                    )
                    nc.vector.tensor_copy(
                        lrep[:].rearrange("p (m w) -> p w m", w=8)[:, w, :], pps[:]
                    )
                s2 = sp.tile([128, 8 * NT], dt.float32, tag="s2")
                nc.scalar.activation(s2[:], lrep[:], AF.Sigmoid)
                ps2 = sp.tile([128, 8 * NT], dt.float32, tag="ps2")
                nc.vector.tensor_scalar_mul(ps2[:], s2[:], 4096.0)
                pi2 = _floor(nc, sp, ps2, "2")
                # single block-start index per t: clamp(p_int, 3, 4092) - 3
                idxs = sp.tile([128, NT, 8], dt.int16, tag="idxs")
                tmpp = sp.tile([128, 8 * NT], dt.float32, tag="tmpp")
                nc.vector.tensor_scalar(
                    tmpp[:], pi2[:], 3.0, 4092.0, ALU.max, ALU.min
                )
                nc.vector.tensor_scalar(
                    idxs[:].rearrange("p m w -> p (m w)"), tmpp[:], -3.0, None, ALU.add
                )

                # ---- t-partition p values: sigma, p_t, p_int, gauss, mask ----
                sig8 = sp.tile([128, NT], dt.float32, tag="sig8")
                nc.scalar.activation(sig8[:], logits8[:], AF.Sigmoid)
                pt8 = sp.tile([128, NT], dt.float32, tag="pt8")
                nc.vector.tensor_scalar_mul(pt8[:], sig8[:], 4096.0)
                pi8 = _floor(nc, sp, pt8, "8")

                NW = NT * NJ
                pos_all = sp.tile([128, NW], dt.float32, tag="pos_all")
                pos3 = pos_all[:].rearrange("p (m j) -> p m j", j=NJ)
                nc.vector.scalar_tensor_tensor(
                    pos3, pi8[:, :, None].broadcast_to([128, NT, NJ]), 1.0,
                    offst[:].rearrange("p (m j) -> p m j", j=NJ),
                    ALU.bypass, ALU.add,
                )
                dtile = sp.tile([128, NW], dt.float32, tag="dtile")
                nc.vector.scalar_tensor_tensor(
                    dtile[:].rearrange("p (m j) -> p m j", j=NJ),
                    pt8[:, :, None].broadcast_to([128, NT, NJ]), 1.0,
                    pos3, ALU.bypass, ALU.subtract,
                )
                g1 = sp.tile([128, NW], dt.float32, tag="g1")
                nc.scalar.activation(g1[:], dtile[:], AF.Square, scale=float(np.sqrt(2.0) / 3.0))
                gauss = sp.tile([128, NW], dt.float32, tag="gauss")
                nc.scalar.activation(gauss[:], g1[:], AF.Exp, scale=-1.0)
                m1 = sp.tile([128, NW], dt.float32, tag="m1")
                nc.vector.tensor_scalar(m1[:], pos_all[:], 0.0, -1e30, ALU.is_lt, ALU.mult)
                maskb = sp.tile([128, NW], dt.float32, tag="maskb")
                nc.vector.tensor_scalar(maskb[:], pos_all[:], 4095.0, -1e30, ALU.is_gt, ALU.mult)
                nc.vector.tensor_add(maskb[:], maskb[:], m1[:])

                # ---- gathers (one 7-row block descriptor per t) + scores ----
                qwin = qT16[b].copy()
                qwin.ap = bass_rust.VecI64Pair([[QS, S - NJ + 1], [1, NJ * QS]])
                a_all = sp.tile([128, NW], dt.float32, tag="a_all")
                gts = []
                for m in range(NT):
                    gt = gp.tile([128, 1, NJ * QS], dt.float16, tag="gt")
                    nc.gpsimd.dma_gather(
                        gt[:], qwin, idxs[:, m, :], 128, 128, NJ * QS,
                        elem_step=QS, single_packet=False,
                    )
                    gtv = gt[:, 0, :].rearrange("p (j q) -> p j q", j=NJ)
                    gts.append(gtv)
                    for j in range(NJ):
                        junk16 = jp.tile([128, QS], dt.float16, tag="junk16")
                        nc.vector.scalar_tensor_tensor(
                            junk16[:], gtv[:, j, :], 1.0, u16s[m][:],
                            ALU.bypass, ALU.mult,
                            accum_out=a_all[:, m * NJ + j : m * NJ + j + 1],
                        )

                # ---- batched masked softmax * gauss ----
                nc.vector.tensor_add(a_all[:], a_all[:], maskb[:])
                a3 = a_all[:].rearrange("p (m j) -> p m j", j=NJ)
                rmax = sp.tile([128, NT], dt.float32, tag="rmax")
                nc.vector.tensor_reduce(rmax[:, :, None], a3, mybir.AxisListType.X, ALU.max)
                asub = sp.tile([128, NW], dt.float32, tag="asub")
                nc.vector.scalar_tensor_tensor(
                    asub[:].rearrange("p (m j) -> p m j", j=NJ),
                    rmax[:, :, None].broadcast_to([128, NT, NJ]), 1.0,
                    a3, ALU.bypass, ALU.subtract,
                )
                e_all = sp.tile([128, NW], dt.float32, tag="e_all")
                nc.scalar.activation(e_all[:], asub[:], AF.Exp, scale=-1.0)
                rsum = sp.tile([128, NT], dt.float32, tag="rsum")
                nc.vector.tensor_reduce(
                    rsum[:, :, None], e_all[:].rearrange("p (m j) -> p m j", j=NJ),
                    mybir.AxisListType.X, ALU.add,
                )
                rinv = sp.tile([128, NT], dt.float32, tag="rinv")
                nc.vector.reciprocal(rinv[:], rsum[:])
                wt = sp.tile([128, NW], dt.float32, tag="wt")
                nc.vector.scalar_tensor_tensor(
                    wt[:].rearrange("p (m j) -> p m j", j=NJ),
                    rinv[:, :, None].broadcast_to([128, NT, NJ]), 1.0,
                    e_all[:].rearrange("p (m j) -> p m j", j=NJ),
                    ALU.bypass, ALU.mult,
                )
                nc.vector.tensor_mul(wt[:], wt[:], gauss[:])
                wt16 = sp.tile([128, NW], dt.float16, tag="wt16")
                nc.vector.tensor_copy(wt16[:], wt[:])

                # ---- weighted sum via diagonal fp16 matmuls ----
                for m in range(NT):
                    dall = sp.tile([128, NJ * 128], dt.float16, tag="dall")
                    nc.vector.tensor_tensor(
                        dall[:].rearrange("p (j q) -> p j q", j=NJ),
                        id128ht[:, None, :].broadcast_to([128, NJ, 128]),
                        wt16[:, m * NJ : (m + 1) * NJ][:, :, None].broadcast_to([128, NJ, 128]),
                        ALU.mult,
                    )
                    wps = wsp.tile([128, QS], dt.float32, tag="wps", space="PSUM")
                    for j in range(NJ):
                        nc.tensor.matmul(
                            wps[:], dall[:, j * 128 : (j + 1) * 128], gts[m][:, j, :],
                            start=(j == 0), stop=(j == NJ - 1),
                        )
                    outt = op.tile([128, QS], dt.float16, tag="outt")
                    nc.scalar.activation(outt[:], wps[:], AF.Copy)
                    nc.sync.dma_start(out[b, m * 128 : (m + 1) * 128, :], outt[:])

    nc.compile()
    return nc


def _host_prep(q, c_t, W_a, W_p, V_p):
    q = np.asarray(q, dtype=np.float32)
    c_t = np.asarray(c_t, dtype=np.float32)
    W_a = np.asarray(W_a, dtype=np.float32)
    W_p = np.asarray(W_p, dtype=np.float32)
    V_p = np.asarray(V_p, dtype=np.float32)

    qT16 = np.ascontiguousarray(q.transpose(0, 2, 1)).astype(np.float16)
    cT = np.ascontiguousarray(c_t.transpose(0, 2, 1))
    cT1 = cT.astype(np.float16)
    cT2 = (cT - cT1.astype(np.float32)).astype(np.float16)
    wpT = np.ascontiguousarray(W_p.T)
    wp1 = wpT.astype(np.float16)
    wp2 = (wpT - wp1.astype(np.float32)).astype(np.float16)
    wa1 = W_a.astype(np.float16)
    vpr = np.ascontiguousarray(np.tile(V_p.reshape(1, PS), (128, 1)), dtype=np.float32)
    offs = np.tile(np.arange(-3, 4, dtype=np.float32).reshape(1, 1, NJ), (128, NT, 1))
    offs = np.ascontiguousarray(offs.reshape(128, NT * NJ))
    perm8 = np.zeros((128, 8, 128), dtype=np.float32)
    for w in range(8):
        for p in range(128):
            perm8[w * 16 + p % 16, w, p] = 1.0
    id128h = np.eye(128).astype(np.float16)

    consts = dict(wp1=wp1, wp2=wp2, wa1=wa1, vpr=vpr, offs=offs, perm8=perm8,
                  id128h=id128h)
    in_maps = []
    for k in range(NCORE):
        sl = slice(k * BPC, (k + 1) * BPC)
        m = dict(consts)
        m["qT16"] = np.ascontiguousarray(qT16[sl])
        m["cT1"] = np.ascontiguousarray(cT1[sl])
        m["cT2"] = np.ascontiguousarray(cT2[sl])
        in_maps.append(m)
    return in_maps


def kernel(q, c_t, W_a, W_p, V_p):
    global LAST_EXEC_NS, LAST_RES
    if "nc" not in _CACHE:
        _CACHE["nc"] = _build_nc()
    nc = _CACHE["nc"]
    in_maps = _host_prep(q, c_t, W_a, W_p, V_p)
    res = run_bass_kernel_spmd(nc, in_maps, core_ids=list(range(NCORE)))
    LAST_RES = res
    LAST_EXEC_NS = res.exec_time_ns
    outs = [res.results[k]["out"] for k in range(NCORE)]
    return np.concatenate(outs, axis=0).astype(np.float32)



# revision 14
# speedup vs baseline: 1.7896x; 1.1185x over previous
"""LocalAttention1d Trainium2 kernel.

Layout strategy (B=16 sharded over 8 cores, 2 batches/core):
  - p_t chain in ~fp32 precision: h = tanh(c@W_p.T) via fp16x2 split matmuls
    (c = c1+c2 fp16 pair, W likewise; 3 cross terms give ~1e-7 rel accuracy),
    logit = <tanh(h), V_p> via fused DVE multiply-reduce in fp32.
  - windowed gather: p_int -> int16 block-start indices -> SWDGE dma_gather
    of 7 contiguous q^T rows (one 7KB descriptor per t) from DRAM fp16.
  - scores: fused DVE multiply-reduce (fp16) against u = c@W_a.
  - softmax*gauss -> 7 diagonal fp16 matmuls accumulate the weighted sum in
    PSUM (t-partition layout).
  - software pipeline over 4 half-batch stages (2 batches x 2 tile-groups)
    with staggered emission so PE/DVE/GPSIMD queues overlap across stages.
"""

import sys

sys.path.insert(0, "/opt/trn_rl_repo")

import numpy as np

import bass_rust
import concourse.bass as bass
import concourse.tile as tile
from concourse import bacc, mybir
from concourse.bass_utils import run_bass_kernel_spmd

B, T, S, QS, CS, PS, D = 16, 1024, 4096, 512, 512, 512, 3
NCORE = 8
BPC = B // NCORE  # batches per core
NJ = 2 * D + 1  # 7 window positions
NT = T // 128  # 8 t-tiles per batch
NH = 4  # tiles per half-batch stage
NSTG = BPC * 2  # pipeline stages

dt = mybir.dt
AF = mybir.ActivationFunctionType
ALU = mybir.AluOpType

LAST_EXEC_NS = None
LAST_RES = None
_CACHE = {}


def _floor(nc, sp, src, sfx):
    """Exact floor(src) for src >= 0, robust to the cast rounding mode."""
    shp = list(src[:].shape)
    i32 = sp.tile(shp, dt.int32, tag="fli" + sfx)
    nc.vector.tensor_copy(i32[:], src[:])
    cand = sp.tile(shp, dt.float32, tag="flc" + sfx)
    nc.vector.tensor_copy(cand[:], i32[:])
    corr = sp.tile(shp, dt.float32, tag="flx" + sfx)
    nc.vector.scalar_tensor_tensor(
        corr[:], cand[:], 1.0, src[:], ALU.bypass, ALU.is_gt
    )
    res = sp.tile(shp, dt.float32, tag="flr" + sfx)
    nc.vector.tensor_tensor(res[:], cand[:], corr[:], ALU.subtract)
    return res


def _build_nc():
    nc = bacc.Bacc("TRN2", target_bir_lowering=False, debug=False, num_devices=NCORE)

    qT16 = nc.dram_tensor("qT16", [BPC, S, QS], dt.float16, kind="ExternalInput").ap()
    cT1 = nc.dram_tensor("cT1", [BPC, CS, T], dt.float16, kind="ExternalInput").ap()
    cT2 = nc.dram_tensor("cT2", [BPC, CS, T], dt.float16, kind="ExternalInput").ap()
    wp1 = nc.dram_tensor("wp1", [CS, PS], dt.float16, kind="ExternalInput").ap()
    wp2 = nc.dram_tensor("wp2", [CS, PS], dt.float16, kind="ExternalInput").ap()
    wa1 = nc.dram_tensor("wa1", [CS, QS], dt.float16, kind="ExternalInput").ap()
    vpr = nc.dram_tensor("vpr", [128, PS], dt.float32, kind="ExternalInput").ap()
    offs = nc.dram_tensor("offs", [128, NT * NJ], dt.float32, kind="ExternalInput").ap()
    perm8 = nc.dram_tensor("perm8", [128, 8, 128], dt.float32, kind="ExternalInput").ap()
    id128h = nc.dram_tensor("id128h", [128, 128], dt.float16, kind="ExternalInput").ap()
    out = nc.dram_tensor("out", [BPC, T, QS], dt.float16, kind="ExternalOutput").ap()

    with tile.TileContext(nc) as tc:
        import contextlib

        ctx = contextlib.ExitStack()
        with ctx:
            cpool = ctx.enter_context(tc.tile_pool(name="consts", bufs=1))
            ctp = ctx.enter_context(tc.tile_pool(name="ct", bufs=16))
            gp = ctx.enter_context(tc.tile_pool(name="gath", bufs=12))
            up = ctx.enter_context(tc.tile_pool(name="u16", bufs=9))
            sp = ctx.enter_context(tc.tile_pool(name="small", bufs=3))
            dp = ctx.enter_context(tc.tile_pool(name="dall", bufs=2))
            gtp = ctx.enter_context(tc.tile_pool(name="tanh", bufs=2))
            jp = ctx.enter_context(tc.tile_pool(name="junk", bufs=2))
            op = ctx.enter_context(tc.tile_pool(name="outp", bufs=2))
            mmp = ctx.enter_context(tc.tile_pool(name="mm", bufs=2, space="PSUM"))
            wsp = ctx.enter_context(tc.tile_pool(name="ws", bufs=2, space="PSUM"))
            tpp = ctx.enter_context(tc.tile_pool(name="tp", bufs=2, space="PSUM"))

            # ---- constants to SBUF; wp first (first h matmul needs only these)
            wp1t = cpool.tile([128, 4, PS], dt.float16)
            nc.sync.dma_start(wp1t[:], wp1[:].rearrange("(k p) n -> p k n", p=128))
            wp2t = cpool.tile([128, 4, PS], dt.float16)
            nc.sync.dma_start(wp2t[:], wp2[:].rearrange("(k p) n -> p k n", p=128))

            # c loads on the scalar queue so they overlap const loads
            all_ct1s, all_ct2s = [], []
            for b in range(BPC):
                ct1s, ct2s = [], []
                for k in range(4):
                    c1t = ctp.tile([128, T], dt.float16, tag="ct1")
                    nc.scalar.dma_start(c1t[:], cT1[b, k * 128 : (k + 1) * 128, :])
                    ct1s.append(c1t)
                for k in range(4):
                    c2t = ctp.tile([128, T], dt.float16, tag="ct2")
                    nc.scalar.dma_start(c2t[:], cT2[b, k * 128 : (k + 1) * 128, :])
                    ct2s.append(c2t)
                all_ct1s.append(ct1s)
                all_ct2s.append(ct2s)

            wa1t = cpool.tile([128, 4, QS], dt.float16)
            nc.sync.dma_start(wa1t[:], wa1[:].rearrange("(k p) n -> p k n", p=128))
            vprt = cpool.tile([128, PS], dt.float32)
            nc.sync.dma_start(vprt[:], vpr[:])
            offst = cpool.tile([128, NT * NJ], dt.float32)
            nc.sync.dma_start(offst[:], offs[:])
            perm8t = cpool.tile([128, 8, 128], dt.float32)
            nc.sync.dma_start(perm8t[:], perm8[:])
            id128ht = cpool.tile([128, 128], dt.float16)
            nc.sync.dma_start(id128ht[:], id128h[:])

            def chunk(t, k):
                return t[:, k, :]

            qwins = []
            for b in range(BPC):
                qw = qT16[b].copy()
                qw.ap = bass_rust.VecI64Pair([[QS, S - NJ + 1], [1, NJ * QS]])
                qwins.append(qw)

            NW = NH * NJ  # 28 window slots per half
            st = [dict() for _ in range(NSTG)]

            def tiles(s):
                b, mg = divmod(s, 2)
                return b, [mg * NH + i for i in range(NH)]

            def phase_A(s):
                """h matmuls + tanh + logit + perm/idx chain for one half."""
                b, ms = tiles(s)
                ct1s, ct2s = all_ct1s[b], all_ct2s[b]
                lg = sp.tile([128, NH], dt.float32, tag="lg")
                st[s]["lg"] = lg
                for i, m in enumerate(ms):
                    hps = mmp.tile([128, PS], dt.float32, tag="hps", space="PSUM")
                    nmm = 0
                    terms = [
                        (ct1s[k][:, m * 128 : (m + 1) * 128], chunk(w, k))
                        for k in range(4)
                        for w in (wp1t, wp2t)
                    ] + [
                        (ct2s[k][:, m * 128 : (m + 1) * 128], chunk(wp1t, k))
                        for k in range(4)
                    ]
                    for lhs, rhs in terms:
                        nc.tensor.matmul(hps[:], lhs, rhs, start=(nmm == 0), stop=(nmm == 11))
                        nmm += 1
                    g = gtp.tile([128, PS], dt.float32, tag="g")
                    nc.scalar.activation(g[:], hps[:], AF.Tanh)
                    junkf = jp.tile([128, PS], dt.float16, tag="junkf")
                    nc.vector.scalar_tensor_tensor(
                        junkf[:], g[:], 1.0, vprt[:], ALU.bypass, ALU.mult,
                        accum_out=lg[:, i : i + 1],
                    )
                # wrapped-16 layout: lrep[p, (i w)] = lg[w*16 + p%16, i]
                lrep = sp.tile([128, NH * 8], dt.float32, tag="lrep")
                for w in range(8):
                    pps = tpp.tile([128, NH], dt.float32, tag="pps", space="PSUM")
                    nc.tensor.matmul(pps[:], perm8t[:, w, :], lg[:], start=True, stop=True)
                    nc.vector.tensor_copy(
                        lrep[:].rearrange("p (m w) -> p w m", w=8)[:, w, :], pps[:]
                    )
                s2 = sp.tile([128, NH * 8], dt.float32, tag="s2")
                nc.scalar.activation(s2[:], lrep[:], AF.Sigmoid)
                ps2 = sp.tile([128, NH * 8], dt.float32, tag="ps2")
                nc.vector.tensor_scalar_mul(ps2[:], s2[:], 4096.0)
                pi2 = _floor(nc, sp, ps2, "2")
                idxs = sp.tile([128, NH, 8], dt.int16, tag="idxs")
                tmpp = sp.tile([128, NH * 8], dt.float32, tag="tmpp")
                nc.vector.tensor_scalar(tmpp[:], pi2[:], 3.0, 4092.0, ALU.max, ALU.min)
                nc.vector.tensor_scalar(
                    idxs[:].rearrange("p m w -> p (m w)"), tmpp[:], -3.0, None, ALU.add
                )
                st[s]["idxs"] = idxs

            def phase_B(s):
                """gathers (gpsimd queue) + t-layout gauss/mask prep."""
                b, ms = tiles(s)
                idxs = st[s]["idxs"]
                gts = []
                for i, m in enumerate(ms):
                    gt = gp.tile([128, 1, NJ * QS], dt.float16, tag="gt")
                    nc.gpsimd.dma_gather(
                        gt[:], qwins[b], idxs[:, i, :], 128, 128, NJ * QS,
                        elem_step=QS, single_packet=False,
                    )
                    gts.append(gt[:, 0, :].rearrange("p (j q) -> p j q", j=NJ))
                st[s]["gts"] = gts

                lg = st[s]["lg"]
                sig8 = sp.tile([128, NH], dt.float32, tag="sig8")
                nc.scalar.activation(sig8[:], lg[:], AF.Sigmoid)
                pt8 = sp.tile([128, NH], dt.float32, tag="pt8")
                nc.vector.tensor_scalar_mul(pt8[:], sig8[:], 4096.0)
                pi8 = _floor(nc, sp, pt8, "8")

                pos_all = sp.tile([128, NW], dt.float32, tag="pos_all")
                pos3 = pos_all[:].rearrange("p (m j) -> p m j", j=NJ)
                nc.vector.scalar_tensor_tensor(
                    pos3, pi8[:, :, None].broadcast_to([128, NH, NJ]), 1.0,
                    offst[:].rearrange("p (m j) -> p m j", j=NJ)[:, :NH, :],
                    ALU.bypass, ALU.add,
                )
                dtile = sp.tile([128, NW], dt.float32, tag="dtile")
                nc.vector.scalar_tensor_tensor(
                    dtile[:].rearrange("p (m j) -> p m j", j=NJ),
                    pt8[:, :, None].broadcast_to([128, NH, NJ]), 1.0,
                    pos3, ALU.bypass, ALU.subtract,
                )
                g1 = sp.tile([128, NW], dt.float32, tag="g1")
                nc.scalar.activation(g1[:], dtile[:], AF.Square, scale=float(np.sqrt(2.0) / 3.0))
                gauss = sp.tile([128, NW], dt.float32, tag="gauss")
                nc.scalar.activation(gauss[:], g1[:], AF.Exp, scale=-1.0)
                m1 = sp.tile([128, NW], dt.float32, tag="m1")
                nc.vector.tensor_scalar(m1[:], pos_all[:], 0.0, -1e30, ALU.is_lt, ALU.mult)
                maskb = sp.tile([128, NW], dt.float32, tag="maskb")
                nc.vector.tensor_scalar(maskb[:], pos_all[:], 4095.0, -1e30, ALU.is_gt, ALU.mult)
                nc.vector.tensor_add(maskb[:], maskb[:], m1[:])
                st[s]["gauss"] = gauss
                st[s]["maskb"] = maskb

            def phase_C(s):
                """u = c1 @ W_a for the half (PE, overlaps gather DMA)."""
                b, ms = tiles(s)
                ct1s = all_ct1s[b]
                u16s = []
                for m in ms:
                    ups = mmp.tile([128, QS], dt.float32, tag="ups", space="PSUM")
                    for k in range(4):
                        nc.tensor.matmul(
                            ups[:], ct1s[k][:, m * 128 : (m + 1) * 128], chunk(wa1t, k),
                            start=(k == 0), stop=(k == 3),
                        )
                    u16 = up.tile([128, QS], dt.float16, tag="u16")
                    nc.scalar.activation(u16[:], ups[:], AF.Copy)
                    u16s.append(u16)
                st[s]["u16s"] = u16s

            def phase_D(s):
                """scores: fused multiply-reduce per (tile, j)."""
                gts, u16s = st[s]["gts"], st[s]["u16s"]
                a_h = sp.tile([128, NW], dt.float32, tag="a_h")
                for i in range(NH):
                    for j in range(NJ):
                        junk16 = jp.tile([128, QS], dt.float16, tag="junk16")
                        nc.vector.scalar_tensor_tensor(
                            junk16[:], gts[i][:, j, :], 1.0, u16s[i][:],
                            ALU.bypass, ALU.mult,
                            accum_out=a_h[:, i * NJ + j : i * NJ + j + 1],
                        )
                st[s]["a_h"] = a_h

            def phase_E(s):
                """masked softmax * gauss -> fp16 diag weights."""
                a_h, gauss, maskb = st[s]["a_h"], st[s]["gauss"], st[s]["maskb"]
                nc.vector.tensor_add(a_h[:], a_h[:], maskb[:])
                a3 = a_h[:].rearrange("p (m j) -> p m j", j=NJ)
                rmax = sp.tile([128, NH], dt.float32, tag="rmax")
                nc.vector.tensor_reduce(rmax[:, :, None], a3, mybir.AxisListType.X, ALU.max)
                asub = sp.tile([128, NW], dt.float32, tag="asub")
                nc.vector.scalar_tensor_tensor(
                    asub[:].rearrange("p (m j) -> p m j", j=NJ),
                    rmax[:, :, None].broadcast_to([128, NH, NJ]), 1.0,
                    a3, ALU.bypass, ALU.subtract,
                )
                e_h = sp.tile([128, NW], dt.float32, tag="e_h")
                nc.scalar.activation(e_h[:], asub[:], AF.Exp, scale=-1.0)
                rsum = sp.tile([128, NH], dt.float32, tag="rsum")
                nc.vector.tensor_reduce(
                    rsum[:, :, None], e_h[:].rearrange("p (m j) -> p m j", j=NJ),
                    mybir.AxisListType.X, ALU.add,
                )
                rinv = sp.tile([128, NH], dt.float32, tag="rinv")
                nc.vector.reciprocal(rinv[:], rsum[:])
                wt = sp.tile([128, NW], dt.float32, tag="wt")
                nc.vector.scalar_tensor_tensor(
                    wt[:].rearrange("p (m j) -> p m j", j=NJ),
                    rinv[:, :, None].broadcast_to([128, NH, NJ]), 1.0,
                    e_h[:].rearrange("p (m j) -> p m j", j=NJ),
                    ALU.bypass, ALU.mult,
                )
                nc.vector.tensor_mul(wt[:], wt[:], st[s]["gauss"][:])
                wt16 = sp.tile([128, NW], dt.float16, tag="wt16")
                nc.vector.tensor_copy(wt16[:], wt[:])
                # diag tiles for the weighted-sum matmuls (one fused op)
                dall = dp.tile([128, NH, NJ, 128], dt.float16, tag="dall")
                nc.vector.tensor_tensor(
                    dall[:],
                    id128ht[:, None, None, :].broadcast_to([128, NH, NJ, 128]),
                    wt16[:].rearrange("p (m j) -> p m j", j=NJ)[:, :, :, None]
                    .broadcast_to([128, NH, NJ, 128]),
                    ALU.mult,
                )
                st[s]["dall"] = dall

            def phase_F(s):
                """weighted sum via diagonal fp16 matmuls + store."""
                b, ms = tiles(s)
                gts, dall = st[s]["gts"], st[s]["dall"]
                for i, m in enumerate(ms):
                    wps = wsp.tile([128, QS], dt.float32, tag="wps", space="PSUM")
                    for j in range(NJ):
                        nc.tensor.matmul(
                            wps[:], dall[:, i, j, :], gts[i][:, j, :],
                            start=(j == 0), stop=(j == NJ - 1),
                        )
                    outt = op.tile([128, QS], dt.float16, tag="outt")
                    nc.scalar.activation(outt[:], wps[:], AF.Copy)
                    nc.sync.dma_start(out[b, m * 128 : (m + 1) * 128, :], outt[:])

            # ---- staggered emission: overlap stages across engine queues ----
            sched = []
            for s in range(NSTG):
                sched += [(phase_A, s), (phase_B, s), (phase_C, s)]
                if s >= 1:
                    sched += [(phase_D, s - 1), (phase_E, s - 1)]
                if s >= 2:
                    sched += [(phase_F, s - 2)]
            sched += [(phase_D, NSTG - 1), (phase_E, NSTG - 1)]
            sched += [(phase_F, NSTG - 2), (phase_F, NSTG - 1)]
            for fn, s in sched:
                fn(s)

    nc.compile()
    return nc


def _host_prep(q, c_t, W_a, W_p, V_p):
    q = np.asarray(q, dtype=np.float32)
    c_t = np.asarray(c_t, dtype=np.float32)
    W_a = np.asarray(W_a, dtype=np.float32)
    W_p = np.asarray(W_p, dtype=np.float32)
    V_p = np.asarray(V_p, dtype=np.float32)

    qT16 = np.ascontiguousarray(q.transpose(0, 2, 1)).astype(np.float16)
    cT = np.ascontiguousarray(c_t.transpose(0, 2, 1))
    cT1 = cT.astype(np.float16)
    cT2 = (cT - cT1.astype(np.float32)).astype(np.float16)
    wpT = np.ascontiguousarray(W_p.T)
    wp1 = wpT.astype(np.float16)
    wp2 = (wpT - wp1.astype(np.float32)).astype(np.float16)
    wa1 = W_a.astype(np.float16)
    vpr = np.ascontiguousarray(np.tile(V_p.reshape(1, PS), (128, 1)), dtype=np.float32)
    offs = np.tile(np.arange(-3, 4, dtype=np.float32).reshape(1, 1, NJ), (128, NT, 1))
    offs = np.ascontiguousarray(offs.reshape(128, NT * NJ))
    perm8 = np.zeros((128, 8, 128), dtype=np.float32)
    for w in range(8):
        for p in range(128):
            perm8[w * 16 + p % 16, w, p] = 1.0
    id128h = np.eye(128).astype(np.float16)

    consts = dict(wp1=wp1, wp2=wp2, wa1=wa1, vpr=vpr, offs=offs, perm8=perm8,
                  id128h=id128h)
    in_maps = []
    for k in range(NCORE):
        sl = slice(k * BPC, (k + 1) * BPC)
        m = dict(consts)
        m["qT16"] = np.ascontiguousarray(qT16[sl])
        m["cT1"] = np.ascontiguousarray(cT1[sl])
        m["cT2"] = np.ascontiguousarray(cT2[sl])
        in_maps.append(m)
    return in_maps


def kernel(q, c_t, W_a, W_p, V_p):
    global LAST_EXEC_NS, LAST_RES
    if "nc" not in _CACHE:
        _CACHE["nc"] = _build_nc()
    nc = _CACHE["nc"]
    in_maps = _host_prep(q, c_t, W_a, W_p, V_p)
    res = run_bass_kernel_spmd(nc, in_maps, core_ids=list(range(NCORE)))
    LAST_RES = res
    LAST_EXEC_NS = res.exec_time_ns
    outs = [res.results[k]["out"] for k in range(NCORE)]
    return np.concatenate(outs, axis=0).astype(np.float32)
